# revision 6
# baseline (speedup 1.0000x reference)
"""Trainium2 Bass kernel for nn_MultiHeadAttention_89232240541956.

Computes, for B=8, S=4096, H=1024, ATTN=1024, EXT=1152:
    x_ext = [h | broadcast(g) | l]                       [B, S, 1152]
    q = relu(x_ext @ Wq + bq); k = relu(x_ext @ Wk + bk) [B, S, 1024]
    scores = sum(q * k, -1) / 32, masked to -1e9 where mask == 1

Sharding: data-parallel over batch — core b owns batch b.

Device-side work is just the two big projections + fused relu/mul/reduce:
  - v (Wv, bv) is dead code in the reference's early-return path — skipped.
  - g @ Wq[1024:1088] is constant over seq for a batch — folded into the
    bias on the host, so the device contracts over 1024 (h) + 64 (l) only.
  - The bias itself is folded into the matmul as one extra contraction row
    (ones-row in x^T against a bias-row in W), so PSUM holds the complete
    pre-activation after 9 accumulating matmuls per 512-wide bank.
  - Host pre-transposes to x^T and pre-casts to bf16, so the contraction
    dim lands on SBUF partitions with no on-device transposes.
  - 1/sqrt(1024) is folded into the fused multiply+reduce (DVE
    tensor_tensor_reduce); masking happens on the host during unshard.
"""

import numpy as np
import ml_dtypes

B, S, H, LOC = 8, 4096, 1024, 64
ATTN = 1024
KX = H + LOC + 1          # 1089 contraction rows: h | l | ones
NE = 8                    # full 128-row h chunks
SBLK = 512                # seq columns per DMA block
NBLK = S // SBLK          # 8
NT = SBLK // 128          # 4 seq tiles (128 tokens) per block
NCOL = S // 128           # 32 output columns

BF16 = ml_dtypes.bfloat16

_CACHE = {}


def _build_nc():
    import concourse.bass as bass
    import concourse.mybir as mybir
    import concourse.tile as tile
    from concourse import bacc

    dt = mybir.dt
    nc = bacc.Bacc(None, target_bir_lowering=False)
    xT = nc.dram_tensor("xT", [KX, S], dt.bfloat16, kind="ExternalInput")
    wq = nc.dram_tensor("wq", [KX, ATTN], dt.bfloat16, kind="ExternalInput")
    wk = nc.dram_tensor("wk", [KX, ATTN], dt.bfloat16, kind="ExternalInput")
    out = nc.dram_tensor("out", [128, NCOL], dt.float32, kind="ExternalOutput")

    with tile.TileContext(nc) as tc:
        with (
            tc.tile_pool(name="wpool", bufs=1) as wpool,
            tc.tile_pool(name="xpool", bufs=2) as xpool,
            tc.tile_pool(name="epool", bufs=2) as epool,
            tc.tile_pool(name="opool", bufs=1) as opool,
            tc.tile_pool(name="psum", bufs=2, space="PSUM") as psum,
        ):
            # Replicated weights: [128, e, n] with contraction chunk e on rows.
            wq_sb = wpool.tile([128, NE, ATTN], dt.bfloat16, tag="wq")
            wk_sb = wpool.tile([128, NE, ATTN], dt.bfloat16, tag="wk")
            for e in range(NE):
                nc.sync.dma_start(wq_sb[:, e, :], wq[e * 128 : (e + 1) * 128, :])
                nc.sync.dma_start(wk_sb[:, e, :], wk[e * 128 : (e + 1) * 128, :])
            # l rows + bias row (contraction chunk of 65).
            wql_sb = wpool.tile([KX - NE * 128, ATTN], dt.bfloat16, tag="wql")
            nc.sync.dma_start(wql_sb[:], wq[NE * 128 :, :])
            wkl_sb = wpool.tile([KX - NE * 128, ATTN], dt.bfloat16, tag="wkl")
            nc.sync.dma_start(wkl_sb[:], wk[NE * 128 :, :])

            score_sb = opool.tile([128, NCOL], dt.float32, tag="score")

            for blk in range(NBLK):
                c0 = blk * SBLK
                xh = xpool.tile([128, NE, SBLK], dt.bfloat16, tag="xh")
                for e in range(NE):
                    nc.sync.dma_start(
                        xh[:, e, :], xT[e * 128 : (e + 1) * 128, c0 : c0 + SBLK]
                    )
                xl = xpool.tile([KX - NE * 128, SBLK], dt.bfloat16, tag="xl")
                nc.sync.dma_start(xl[:], xT[NE * 128 :, c0 : c0 + SBLK])

                for t in range(NT):
                    s0 = t * 128
                    psq = psum.tile([128, ATTN], dt.float32, tag="psq")
                    psk = psum.tile([128, ATTN], dt.float32, tag="psk")
                    for e in range(NE):
                        lhs = xh[:, e, s0 : s0 + 128]
                        for nh in range(2):
                            n0 = nh * 512
                            nc.tensor.matmul(
                                psq[:, n0 : n0 + 512],
                                lhs,
                                wq_sb[:, e, n0 : n0 + 512],
                                start=(e == 0),
                                stop=False,
                            )
                            nc.tensor.matmul(
                                psk[:, n0 : n0 + 512],
                                lhs,
                                wk_sb[:, e, n0 : n0 + 512],
                                start=(e == 0),
                                stop=False,
                            )
                    lhs_l = xl[:, s0 : s0 + 128]
                    for nh in range(2):
                        n0 = nh * 512
                        nc.tensor.matmul(
                            psq[:, n0 : n0 + 512],
                            lhs_l,
                            wql_sb[:, n0 : n0 + 512],
                            start=False,
                            stop=True,
                        )
                        nc.tensor.matmul(
                            psk[:, n0 : n0 + 512],
                            lhs_l,
                            wkl_sb[:, n0 : n0 + 512],
                            start=False,
                            stop=True,
                        )

                    qsb = epool.tile([128, ATTN], dt.bfloat16, tag="qsb")
                    nc.scalar.activation(
                        qsb[:], psq[:], mybir.ActivationFunctionType.Relu
                    )
                    ksb = epool.tile([128, ATTN], dt.bfloat16, tag="ksb")
                    nc.scalar.activation(
                        ksb[:], psk[:], mybir.ActivationFunctionType.Relu
                    )
                    prod = epool.tile([128, ATTN], dt.bfloat16, tag="prod")
                    nc.vector.tensor_mul(prod[:], qsb[:], ksb[:])
                    cpy = epool.tile([128, ATTN], dt.bfloat16, tag="cpy")
                    col = blk * NT + t
                    nc.scalar.activation(
                        cpy[:],
                        prod[:],
                        mybir.ActivationFunctionType.Copy,
                        scale=1.0 / 32.0,
                        accum_out=score_sb[:, col : col + 1],
                    )

            nc.sync.dma_start(out[:], score_sb[:])

    nc.compile()
    return nc


def _get_nc():
    if "nc" not in _CACHE:
        _CACHE["nc"] = _build_nc()
    return _CACHE["nc"]


def prep_in_maps(h, mask, g, l, Wq, bq, Wk, bk, Wv=None, bv=None):
    h = np.asarray(h, dtype=np.float32)
    g = np.asarray(g, dtype=np.float32)
    l_ = np.asarray(l, dtype=np.float32)
    Wq = np.asarray(Wq, dtype=np.float32)
    bq = np.asarray(bq, dtype=np.float32)
    Wk = np.asarray(Wk, dtype=np.float32)
    bk = np.asarray(bk, dtype=np.float32)

    # Fold the per-batch g contribution into the bias (fp32 on host).
    bq_eff = bq[None, :] + g @ Wq[H : H + LOC]          # [B, ATTN]
    bk_eff = bk[None, :] + g @ Wk[H : H + LOC]

    wq_base = np.empty((KX, ATTN), dtype=BF16)
    wq_base[:H] = Wq[:H]
    wq_base[H : H + LOC] = Wq[H + LOC :]                # l rows
    wk_base = np.empty((KX, ATTN), dtype=BF16)
    wk_base[:H] = Wk[:H]
    wk_base[H : H + LOC] = Wk[H + LOC :]

    in_maps = []
    for b in range(B):
        xT = np.empty((KX, S), dtype=BF16)
        xT[:H] = h[b].T
        xT[H : H + LOC] = l_[b].T
        xT[H + LOC] = np.ones((S,), dtype=BF16)
        wq_b = wq_base.copy()
        wq_b[H + LOC] = bq_eff[b]
        wk_b = wk_base.copy()
        wk_b[H + LOC] = bk_eff[b]
        in_maps.append({"xT": xT, "wq": wq_b, "wk": wk_b})
    return in_maps


def kernel(h, mask, g, l, Wq, bq, Wk, bk, Wv=None, bv=None):
    from concourse.bass_utils import run_bass_kernel_spmd

    mask = np.asarray(mask)
    in_maps = prep_in_maps(h, mask, g, l, Wq, bq, Wk, bk)

    nc = _get_nc()
    res = run_bass_kernel_spmd(nc, in_maps, core_ids=list(range(B)), trace=False)

    scores = np.empty((B, S), dtype=np.float32)
    for b in range(B):
        scores[b] = res.results[b]["out"].T.reshape(S)
    return np.where(mask == 1, np.float32(-1e9), scores).astype(np.float32)


# revision 7
# speedup vs baseline: 1.0401x; 1.0401x over previous
"""Trainium2 Bass kernel for nn_MultiHeadAttention_89232240541956.

Computes, for B=8, S=4096, H=1024, ATTN=1024, EXT=1152:
    x_ext = [h | broadcast(g) | l]                       [B, S, 1152]
    q = relu(x_ext @ Wq + bq); k = relu(x_ext @ Wk + bk) [B, S, 1024]
    scores = sum(q * k, -1) / 32, masked to -1e9 where mask == 1

Sharding: data-parallel over batch — core b owns batch b.

Device-side work is just the two big projections + fused relu/mul/reduce:
  - v (Wv, bv) is dead code in the reference's early-return path — skipped.
  - g @ Wq[1024:1088] is constant over seq for a batch — folded into the
    bias on the host, so the device contracts over 1024 (h) + 64 (l) only.
  - The bias itself is folded into the matmul as one extra contraction row
    (ones-row in x^T against a bias-row in W), so PSUM holds the complete
    pre-activation after 9 accumulating matmuls per 512-wide bank.
  - Host pre-transposes to x^T and pre-casts to bf16, so the contraction
    dim lands on SBUF partitions with no on-device transposes.
  - 1/sqrt(1024) is folded into the fused multiply+reduce (DVE
    tensor_tensor_reduce); masking happens on the host during unshard.
"""

import numpy as np
import ml_dtypes

B, S, H, LOC = 8, 4096, 1024, 64
ATTN = 1024
KX = H + LOC + 1          # 1089 contraction rows: h | l | ones
NE = 8                    # full 128-row h chunks
SBLK = 512                # seq columns per DMA block
NBLK = S // SBLK          # 8
NT = SBLK // 128          # 4 seq tiles (128 tokens) per block
NCOL = S // 128           # 32 output columns

BF16 = ml_dtypes.bfloat16

_CACHE = {}


def _build_nc():
    import concourse.bass as bass
    import concourse.mybir as mybir
    import concourse.tile as tile
    from concourse import bacc

    dt = mybir.dt
    nc = bacc.Bacc(None, target_bir_lowering=False)
    xT = nc.dram_tensor("xT", [KX, S], dt.bfloat16, kind="ExternalInput")
    wq = nc.dram_tensor("wq", [KX, ATTN], dt.bfloat16, kind="ExternalInput")
    wk = nc.dram_tensor("wk", [KX, ATTN], dt.bfloat16, kind="ExternalInput")
    out = nc.dram_tensor("out", [128, NCOL], dt.float32, kind="ExternalOutput")

    with tile.TileContext(nc) as tc:
        with (
            tc.tile_pool(name="wpool", bufs=1) as wpool,
            tc.tile_pool(name="xpool", bufs=2) as xpool,
            tc.tile_pool(name="epool", bufs=2) as epool,
            tc.tile_pool(name="opool", bufs=1) as opool,
            tc.tile_pool(name="psum", bufs=2, space="PSUM") as psum,
        ):
            # Replicated weights: [128, e, n] with contraction chunk e on rows.
            # Emission order interleaves block-0 x chunks with the weight
            # chunks so the first seq tile's accumulation group has chunk e
            # available as early as possible (PE starts ~20us sooner).
            wq_sb = wpool.tile([128, NE, ATTN], dt.bfloat16, tag="wq")
            wk_sb = wpool.tile([128, NE, ATTN], dt.bfloat16, tag="wk")
            wql_sb = wpool.tile([KX - NE * 128, ATTN], dt.bfloat16, tag="wql")
            wkl_sb = wpool.tile([KX - NE * 128, ATTN], dt.bfloat16, tag="wkl")
            xh0 = xpool.tile([128, NE, SBLK], dt.bfloat16, tag="xh")
            for e in range(NE):
                nc.sync.dma_start(wq_sb[:, e, :], wq[e * 128 : (e + 1) * 128, :])
                nc.sync.dma_start(wk_sb[:, e, :], wk[e * 128 : (e + 1) * 128, :])
                nc.sync.dma_start(xh0[:, e, :], xT[e * 128 : (e + 1) * 128, 0:SBLK])
            nc.sync.dma_start(wql_sb[:], wq[NE * 128 :, :])
            nc.sync.dma_start(wkl_sb[:], wk[NE * 128 :, :])
            xl0 = xpool.tile([KX - NE * 128, SBLK], dt.bfloat16, tag="xl")
            nc.sync.dma_start(xl0[:], xT[NE * 128 :, 0:SBLK])

            score_sb = opool.tile([128, NCOL], dt.float32, tag="score")

            for blk in range(NBLK):
                c0 = blk * SBLK
                if blk == 0:
                    xh = xh0
                    xl = xl0
                else:
                    xh = xpool.tile([128, NE, SBLK], dt.bfloat16, tag="xh")
                    for e in range(NE):
                        nc.sync.dma_start(
                            xh[:, e, :], xT[e * 128 : (e + 1) * 128, c0 : c0 + SBLK]
                        )
                    xl = xpool.tile([KX - NE * 128, SBLK], dt.bfloat16, tag="xl")
                    nc.sync.dma_start(xl[:], xT[NE * 128 :, c0 : c0 + SBLK])

                for t in range(NT):
                    s0 = t * 128
                    psq = psum.tile([128, ATTN], dt.float32, tag="psq")
                    psk = psum.tile([128, ATTN], dt.float32, tag="psk")
                    for e in range(NE):
                        lhs = xh[:, e, s0 : s0 + 128]
                        for nh in range(2):
                            n0 = nh * 512
                            nc.tensor.matmul(
                                psq[:, n0 : n0 + 512],
                                lhs,
                                wq_sb[:, e, n0 : n0 + 512],
                                start=(e == 0),
                                stop=False,
                            )
                            nc.tensor.matmul(
                                psk[:, n0 : n0 + 512],
                                lhs,
                                wk_sb[:, e, n0 : n0 + 512],
                                start=(e == 0),
                                stop=False,
                            )
                    lhs_l = xl[:, s0 : s0 + 128]
                    for nh in range(2):
                        n0 = nh * 512
                        nc.tensor.matmul(
                            psq[:, n0 : n0 + 512],
                            lhs_l,
                            wql_sb[:, n0 : n0 + 512],
                            start=False,
                            stop=True,
                        )
                        nc.tensor.matmul(
                            psk[:, n0 : n0 + 512],
                            lhs_l,
                            wkl_sb[:, n0 : n0 + 512],
                            start=False,
                            stop=True,
                        )

                    qsb = epool.tile([128, ATTN], dt.bfloat16, tag="qsb")
                    nc.scalar.activation(
                        qsb[:], psq[:], mybir.ActivationFunctionType.Relu
                    )
                    ksb = epool.tile([128, ATTN], dt.bfloat16, tag="ksb")
                    nc.scalar.activation(
                        ksb[:], psk[:], mybir.ActivationFunctionType.Relu
                    )
                    prod = epool.tile([128, ATTN], dt.bfloat16, tag="prod")
                    nc.vector.tensor_mul(prod[:], qsb[:], ksb[:])
                    cpy = epool.tile([128, ATTN], dt.bfloat16, tag="cpy")
                    col = blk * NT + t
                    nc.scalar.activation(
                        cpy[:],
                        prod[:],
                        mybir.ActivationFunctionType.Copy,
                        scale=1.0 / 32.0,
                        accum_out=score_sb[:, col : col + 1],
                    )

            nc.sync.dma_start(out[:], score_sb[:])

    nc.compile()
    return nc


def _get_nc():
    if "nc" not in _CACHE:
        _CACHE["nc"] = _build_nc()
    return _CACHE["nc"]


def prep_in_maps(h, mask, g, l, Wq, bq, Wk, bk, Wv=None, bv=None):
    h = np.asarray(h, dtype=np.float32)
    g = np.asarray(g, dtype=np.float32)
    l_ = np.asarray(l, dtype=np.float32)
    Wq = np.asarray(Wq, dtype=np.float32)
    bq = np.asarray(bq, dtype=np.float32)
    Wk = np.asarray(Wk, dtype=np.float32)
    bk = np.asarray(bk, dtype=np.float32)

    # Fold the per-batch g contribution into the bias (fp32 on host).
    bq_eff = bq[None, :] + g @ Wq[H : H + LOC]          # [B, ATTN]
    bk_eff = bk[None, :] + g @ Wk[H : H + LOC]

    wq_base = np.empty((KX, ATTN), dtype=BF16)
    wq_base[:H] = Wq[:H]
    wq_base[H : H + LOC] = Wq[H + LOC :]                # l rows
    wk_base = np.empty((KX, ATTN), dtype=BF16)
    wk_base[:H] = Wk[:H]
    wk_base[H : H + LOC] = Wk[H + LOC :]

    in_maps = []
    for b in range(B):
        xT = np.empty((KX, S), dtype=BF16)
        xT[:H] = h[b].T
        xT[H : H + LOC] = l_[b].T
        xT[H + LOC] = np.ones((S,), dtype=BF16)
        wq_b = wq_base.copy()
        wq_b[H + LOC] = bq_eff[b]
        wk_b = wk_base.copy()
        wk_b[H + LOC] = bk_eff[b]
        in_maps.append({"xT": xT, "wq": wq_b, "wk": wk_b})
    return in_maps


def kernel(h, mask, g, l, Wq, bq, Wk, bk, Wv=None, bv=None):
    from concourse.bass_utils import run_bass_kernel_spmd

    mask = np.asarray(mask)
    in_maps = prep_in_maps(h, mask, g, l, Wq, bq, Wk, bk)

    nc = _get_nc()
    res = run_bass_kernel_spmd(nc, in_maps, core_ids=list(range(B)), trace=False)

    scores = np.empty((B, S), dtype=np.float32)
    for b in range(B):
        scores[b] = res.results[b]["out"].T.reshape(S)
    return np.where(mask == 1, np.float32(-1e9), scores).astype(np.float32)
